# revision 1
# baseline (speedup 1.0000x reference)
"""Trainium2 Bass kernel for a 2-layer LIF spiking network (T=50, B=1024,
784 -> 1024 -> 10), data-parallel over batch across 8 NeuronCores.

Strategy:
  - Layer-1 matmuls (x[t] @ W1.T) have no recurrent dependency: computed in
    bulk on the PE in a "transposed" formulation out = W1 @ x[t].T so the
    hidden dim lands on partitions and layer 2 needs no transpose.
  - fp32 accuracy via 3-pass hi/lo split: x = xh(fp16) + xl(bf16 residual),
    W1*64 = Wh(fp16) + Wl(fp16 residual).  cur1*64 accumulates in PSUM as
    xh@Wh + xl@bf16(Wh) + xh@Wl.  The *64 scale keeps W fp16-splits out of
    the subnormal range; LIF state is simply kept at 64x scale (exact,
    power of two) and the mem2 output is scaled back by 1/64 on write-out.
  - LIF updates run on the vector engine; tiny layer-2 matmuls (K=1024,
    M=10) are chunk-batched (4 timesteps -> N=512) on the PE.
"""

import os
import sys

import numpy as np
import ml_dtypes

sys.path.insert(0, "/opt/trn_rl_repo")

T, B, N_IN, N_HID, N_OUT = 50, 1024, 784, 1024, 10
NCORES = 8
BS = B // NCORES            # batch shard per core = 128
KT, KP = 7, 112             # 784 = 7 k-tiles of 112
HT = N_HID // 128           # 8 hidden tiles
SCALE = 64.0
CHUNK = 4                   # timesteps per layer-2/psum chunk (N = 512)
SUPER = 8                   # timesteps per x-stream DMA window

LAST_RESULT = None          # BassKernelResults of the last run (for test.py)


def _build_bass(b1: float, b2: float):
    import concourse.bass as bass
    from concourse import bacc
    import concourse.mybir as mybir
    import concourse.tile as tile

    f32 = mybir.dt.float32
    f16 = mybir.dt.float16
    bf16 = mybir.dt.bfloat16
    Alu = mybir.AluOpType
    Act = mybir.ActivationFunctionType

    nc = bacc.Bacc("TRN2", target_bir_lowering=False, debug=False,
                   num_devices=NCORES)

    TB = T * BS  # 6400
    xh_d = nc.dram_tensor("xh", [KP, KT, TB], f16, kind="ExternalInput")
    xl_d = nc.dram_tensor("xl", [KP, KT, TB], bf16, kind="ExternalInput")
    w1h_d = nc.dram_tensor("w1h", [HT, KP, KT * 128], f16, kind="ExternalInput")
    w1l_d = nc.dram_tensor("w1l", [HT, KP, KT * 128], f16, kind="ExternalInput")
    w1hb_d = nc.dram_tensor("w1hb", [HT, KP, KT * 128], bf16, kind="ExternalInput")
    w2h_d = nc.dram_tensor("w2h", [128, HT * N_OUT], f16, kind="ExternalInput")
    w2l_d = nc.dram_tensor("w2l", [128, HT * N_OUT], f16, kind="ExternalInput")
    spk_d = nc.dram_tensor("spk2o", [N_OUT, TB], f32, kind="ExternalOutput")
    mem_d = nc.dram_tensor("mem2o", [N_OUT, TB], f32, kind="ExternalOutput")

    # supers: (t0, nsteps)
    supers = []
    t0 = 0
    while t0 < T:
        supers.append((t0, min(SUPER, T - t0)))
        t0 += SUPER

    with tile.TileContext(nc) as tc:
        with (
            tc.tile_pool(name="const", bufs=1) as cpool,
            tc.tile_pool(name="xs", bufs=2) as xpool,
            tc.tile_pool(name="cur", bufs=3) as curpool,
            tc.tile_pool(name="spk", bufs=2) as spkpool,
            tc.tile_pool(name="state", bufs=1) as stpool,
            tc.tile_pool(name="outst", bufs=2) as opool,
            tc.tile_pool(name="ps1", bufs=4, space="PSUM") as ps1pool,
            tc.tile_pool(name="ps2", bufs=2, space="PSUM") as ps2pool,
        ):
            # ---- first x window + weights, split per-k so the first
            # matmul group's operands arrive progressively ----
            n_first = min(SUPER, T)
            xh0 = xpool.tile([KP, KT, n_first * BS], f16, tag="xh")
            xl0 = xpool.tile([KP, KT, n_first * BS], bf16, tag="xl")
            w1h = cpool.tile([KP, HT, KT, 128], f16)
            w1l = cpool.tile([KP, HT, KT, 128], f16)
            w1hb = cpool.tile([KP, HT, KT, 128], bf16)
            w2h = cpool.tile([128, HT * N_OUT], f16)
            w2l = cpool.tile([128, HT * N_OUT], f16)
            for k in range(KT):
                nc.gpsimd.dma_start(xh0[:, k, :],
                                    xh_d[:, k, 0:n_first * BS])
            for k in range(KT):
                nc.gpsimd.dma_start(xl0[:, k, :],
                                    xl_d[:, k, 0:n_first * BS])
            # weights arrive h-tile by h-tile, matching the first chunk's
            # consumption order (one h needs only ~600KB of W)
            for h in range(HT):
                nc.sync.dma_start(w1h[:, h], w1h_d[h])
                nc.sync.dma_start(w1hb[:, h], w1hb_d[h])
                nc.sync.dma_start(w1l[:, h], w1l_d[h])
            nc.sync.dma_start(w2h[:], w2h_d[:])
            nc.sync.dma_start(w2l[:], w2l_d[:])

            # ---- persistent LIF state (kept at 64x scale) ----
            m1 = stpool.tile([128, HT, 128], f32)   # mem1*64, free=(h, b)
            u1 = stpool.tile([128, HT, 128], f32)
            k1 = stpool.tile([128, HT, 128], f16)   # 1 - spike1 (keep mask)
            m2 = stpool.tile([N_OUT, 128], f32)
            u2 = stpool.tile([N_OUT, 128], f32)
            k2 = stpool.tile([N_OUT, 128], f16)
            nc.vector.memset(m1[:], 0.0)
            nc.vector.memset(k1[:], 1.0)
            nc.vector.memset(m2[:], 0.0)
            nc.vector.memset(k2[:], 1.0)

            for si, (s0, nsteps) in enumerate(supers):
                NW = nsteps * BS
                win = slice(s0 * BS, s0 * BS + NW)
                if si == 0:
                    xh, xl = xh0, xl0
                else:
                    xh = xpool.tile([KP, KT, NW], f16, tag="xh")
                    xl = xpool.tile([KP, KT, NW], bf16, tag="xl")
                    nc.gpsimd.dma_start(xh[:], xh_d[:, :, win])
                    nc.gpsimd.dma_start(xl[:], xl_d[:, :, win])

                chunks = []
                c0 = 0
                csize = CHUNK if nsteps > 2 else 1  # tail super: overlap LIF
                while c0 < nsteps:
                    chunks.append((c0, min(csize, nsteps - c0)))
                    c0 += csize

                # ---- layer-1 matmuls, chunk-major so the LIF chain can
                # start as soon as a chunk's last h-tile is evacuated ----
                cur = {}
                for ci, (c0, csz) in enumerate(chunks):
                    cur[ci] = curpool.tile([128, HT, csz * BS], f32,
                                           tag="cur1", name=f"cur1_{ci}")
                for ci, (c0, csz) in enumerate(chunks):
                    for h in range(HT):
                        ps = ps1pool.tile([128, csz * BS], f32, tag="p1",
                                          name=f"p1_{h}_{ci}")
                        npass = 3 * KT
                        ip = 0
                        for wsb, xsb in ((w1h, xh), (w1hb, xl), (w1l, xh)):
                            rhs = xsb[:, :, c0 * BS:(c0 + csz) * BS]
                            for k in range(KT):
                                nc.tensor.matmul(
                                    ps[:], wsb[:, h, k, :], rhs[:, k, :],
                                    start=(ip == 0), stop=(ip == npass - 1))
                                ip += 1
                        nc.scalar.activation(cur[ci][:, h, :], ps[:],
                                             Act.Copy)

                # ---- LIF1 + layer 2 + LIF2, per chunk ----
                for ci, (c0, csz) in enumerate(chunks):
                    NC_ = csz * BS
                    spk1 = spkpool.tile([128, HT, NC_], f16, tag="spk1")
                    for j in range(csz):
                        bs = slice(j * BS, (j + 1) * BS)
                        cj = cur[ci][:, :, bs]
                        # u = b1*m1 + cur ; m1' = u * keep ; spk/keep from m1'
                        nc.vector.scalar_tensor_tensor(
                            u1[:], m1[:], b1, cj, op0=Alu.mult, op1=Alu.add)
                        nc.vector.tensor_tensor(m1[:], u1[:], k1[:],
                                                op=Alu.mult)
                        nc.vector.tensor_scalar(
                            spk1[:, :, bs], m1[:], SCALE, None, op0=Alu.is_gt)
                        nc.vector.tensor_scalar(
                            k1[:], m1[:], SCALE, None, op0=Alu.is_le)
                    # layer 2: cur2.T = (W2*64) @ spk1, 4-way column-packed
                    # (4 concurrent col-groups, each accumulating 2 h-tiles
                    # x {hi,lo}); the 4 partition slabs are summed below.
                    p2 = ps2pool.tile([128, NC_], f32, tag="p2")
                    for cg in range(4):
                        po = 32 * cg
                        ip = 0
                        for h in (2 * cg, 2 * cg + 1):
                            os_ = slice(h * N_OUT, (h + 1) * N_OUT)
                            for wsb in (w2h, w2l):
                                nc.tensor.matmul(
                                    p2[po:po + N_OUT, :], wsb[:, os_],
                                    spk1[:, h, :],
                                    start=(ip == 0), stop=(ip == 3),
                                    tile_position=(0, po))
                                ip += 1
                    c2 = opool.tile([N_OUT, NC_], f32, tag="c2")
                    nc.scalar.activation(c2[:], p2[0:N_OUT, :], Act.Copy)
                    for cg in (1, 2, 3):
                        po = 32 * cg
                        nc.vector.scalar_tensor_tensor(
                            c2[:], p2[po:po + N_OUT, :], 1.0, c2[:],
                            op0=Alu.bypass, op1=Alu.add)
                    spk_st = opool.tile([N_OUT, NC_], f32, tag="spkst")
                    mem_st = opool.tile([N_OUT, NC_], f32, tag="memst")
                    for j in range(csz):
                        bs = slice(j * BS, (j + 1) * BS)
                        nc.vector.scalar_tensor_tensor(
                            u2[:], m2[:], b2, c2[:, bs], op0=Alu.mult,
                            op1=Alu.add)
                        nc.vector.tensor_tensor(m2[:], u2[:], k2[:],
                                                op=Alu.mult)
                        nc.vector.tensor_scalar(
                            spk_st[:, bs], m2[:], SCALE, None, op0=Alu.is_gt)
                        nc.vector.tensor_scalar(
                            k2[:], m2[:], SCALE, None, op0=Alu.is_le)
                        nc.vector.tensor_scalar(
                            mem_st[:, bs], m2[:], 1.0 / SCALE, None,
                            op0=Alu.mult)
                    ow = slice((s0 + c0) * BS, (s0 + c0 + csz) * BS)
                    nc.gpsimd.dma_start(spk_d[:, ow], spk_st[:])
                    nc.gpsimd.dma_start(mem_d[:, ow], mem_st[:])

    nc.compile()
    return nc


def _prep_inputs(x, W1, W2):
    """Host-side layout + hi/lo splits. Returns (per-core xh/xl lists, weights)."""
    f32 = np.float32
    # x: [T, B, N_IN] -> feature-major [N_IN, T, B]
    xt = np.ascontiguousarray(np.transpose(np.asarray(x, f32), (2, 0, 1)))
    xh_full = xt.astype(np.float16)
    xl_full = (xt - xh_full.astype(f32)).astype(ml_dtypes.bfloat16)

    xh_cores, xl_cores = [], []
    for c in range(NCORES):
        bs = slice(c * BS, (c + 1) * BS)
        for src, outl in ((xh_full, xh_cores), (xl_full, xl_cores)):
            a = src[:, :, bs]                       # [784, T, BS]
            a = a.reshape(KT, KP, T * BS)           # [7, 112, 6400]
            a = np.ascontiguousarray(a.transpose(1, 0, 2))  # [112, 7, 6400]
            outl.append(a)

    W1s = np.asarray(W1, f32) * f32(SCALE)          # [N_HID, N_IN]
    W1T = np.ascontiguousarray(W1s.T)               # [784, 1024]
    w1h = W1T.astype(np.float16)
    w1l = (W1T - w1h.astype(f32)).astype(np.float16)
    w1hb = w1h.astype(ml_dtypes.bfloat16)

    def w1_layout(a):
        # [784, 1024] -> [HT, KP, KT*128] with order (h, p, k, m)
        return np.ascontiguousarray(
            a.reshape(KT, KP, HT, 128).transpose(2, 1, 0, 3).reshape(
                HT, KP, KT * 128))

    W2s = np.asarray(W2, f32) * f32(SCALE)          # [N_OUT, N_HID]
    W2T = np.ascontiguousarray(W2s.T)               # [1024, 10]
    w2h = W2T.astype(np.float16)
    w2l = (W2T - w2h.astype(f32)).astype(np.float16)

    def w2_layout(a):
        # [1024, 10] -> [128, HT*10] with free=(h, o)
        return np.ascontiguousarray(
            a.reshape(HT, 128, N_OUT).transpose(1, 0, 2).reshape(
                128, HT * N_OUT))

    weights = {
        "w1h": w1_layout(w1h), "w1l": w1_layout(w1l), "w1hb": w1_layout(w1hb),
        "w2h": w2_layout(w2h), "w2l": w2_layout(w2l),
    }
    return xh_cores, xl_cores, weights



def _ensure_ntff_shim():
    """run_bass_kernel_spmd(trace) imports antenv.axon_hooks, absent in some
    images; install a graceful stand-in so tracing degrades instead of
    crashing."""
    try:
        import antenv.axon_hooks  # noqa: F401
        return
    except Exception:
        pass
    import types
    hook = None
    try:
        from trn_agent_boot.trn_boot import _ntff_profile_via_ctypes
        hook = _ntff_profile_via_ctypes("/opt/axon/libaxon_pjrt.so")
    except Exception:
        hook = None
    mod = types.ModuleType("antenv.axon_hooks")
    mod._hook = hook
    mod.get_axon_ntff_profile_hook = lambda: mod._hook
    mod.set_axon_ntff_profile_hook = lambda h: setattr(mod, "_hook", h)
    sys.modules["antenv.axon_hooks"] = mod


def kernel(x, W1, W2, beta1, beta2):
    global LAST_RESULT
    from concourse.bass_utils import run_bass_kernel_spmd

    _ensure_ntff_shim()

    b1 = float(np.clip(np.float32(beta1), 0.0, 1.0))
    b2 = float(np.clip(np.float32(beta2), 0.0, 1.0))

    xh_cores, xl_cores, weights = _prep_inputs(x, W1, W2)
    nc = _build_bass(b1, b2)

    in_maps = []
    for c in range(NCORES):
        m = {"xh": xh_cores[c], "xl": xl_cores[c]}
        m.update(weights)
        in_maps.append(m)

    res = run_bass_kernel_spmd(nc, in_maps, core_ids=list(range(NCORES)))
    LAST_RESULT = res

    spk_parts, mem_parts = [], []
    for c in range(NCORES):
        r = res.results[c]
        spk_parts.append(
            r["spk2o"].reshape(N_OUT, T, BS).transpose(1, 2, 0))
        mem_parts.append(
            r["mem2o"].reshape(N_OUT, T, BS).transpose(1, 2, 0))
    spk2 = np.ascontiguousarray(np.concatenate(spk_parts, axis=1))
    mem2 = np.ascontiguousarray(np.concatenate(mem_parts, axis=1))
    return spk2, mem2

